# revision 1
# baseline (speedup 1.0000x reference)
"""Trainium2 Bass kernel for nn_ExpertHead: full attention head.

Reference computation (per batch b):
    Q = x Wq^T + bq; K = x Wk^T + bk; V = x Wv^T + bv        [S, D]
    P = softmax(Q K^T / sqrt(D))                              [S, S]
    O = layernorm(P V) -> gelu(exact) -> O Wo^T + bo          [S, D]

Sharding: 8 cores, B=4 batches -> each core handles one half (2048 rows)
of one batch's queries, with full K/V for that batch computed locally
(weights replicated). No collectives. The host rolls x so each core's
queries are always the first QH rows (softmax/PV are permutation
invariant over keys).

Layout strategy (per core):
  - host passes x^T (d-major) in bf16, plus transposed bf16 weights, so
    every matmul contraction dim is already on partitions; no on-chip
    transposes of activations are needed except the post-gelu tiles,
    which use the DMA xbar transpose (bf16).
  - scores are computed TRANSPOSED: S^T[k, q], so exp(S^T) feeds the PV
    matmul directly as the stationary operand.
  - softmax row sums land directly in per-partition-scalar layout via
    matmul(lhsT=exp_block, rhs=ones[128,1]) -> [q, 1]; all 4 q-slice
    sums accumulate in one PSUM bank (one zero region: start only on
    the very first MM, stop on the very last).
  - post-processing (normalize, LN, gelu, transpose, out-proj) of query
    block qb is emitted inside the k-loop of block qb+1 so the PE never
    waits on the DVE/ACT chain.
"""

import numpy as np
import ml_dtypes

import concourse.bass as bass
import concourse.mybir as mybir
import concourse.tile as tile
from concourse import bacc
from concourse.bass_utils import run_bass_kernel_spmd

BF16 = mybir.dt.bfloat16
F32 = mybir.dt.float32
AF = mybir.ActivationFunctionType
ALU = mybir.AluOpType

B, S, D = 4, 4096, 512
P = 128
QH = S // 2          # queries per core
DT = D // P          # 4 contraction tiles of 128
NKT = S // P         # 32 key tiles
NQB = QH // 512      # 4 query blocks of 512
NSB = S // 512       # 8 s blocks of 512
SCALE = float(1.0 / np.sqrt(np.float32(D)))
EPS = 1e-5
N_CORES = 8
POST_KT = 6          # k-iteration of next block where prev post is emitted

TRACE = False
TRACE_KW = {}
last_results = None

_cached_nc = None


def _bcast(ap1d, parts=P):
    """[N] dram AP -> [parts, N] partition-broadcast AP (step 0)."""
    return bass.AP(
        tensor=ap1d.tensor,
        offset=ap1d.offset,
        ap=[[0, parts], list(ap1d.ap[0])],
    )


def _emit_body(nc, tc, ctxpools, handles, rep):
    (xT_h, w_h, bqt_h, bkt_h, bv_h, bo_h, g_h, bb_h, y_h) = handles
    (const, qkv, expp, op, statp, otp, yp) = ctxpools

    # ---- constants / weights into SBUF.
    # DMA queue order matters for PE start latency (SWDGE trigger latency
    # ~1us per transfer): first the xT-half/wv pairs the V projection
    # consumes, then projection-phase biases, then everything else.
    w_sb = {}
    wv_t = const.tile([P, DT, D], BF16, tag="wv", name=f"wv_{rep}")
    w_sb["v"] = wv_t
    xT_sb = const.tile([P, DT, S], BF16, tag="xT", name=f"xT_{rep}")
    H = S // 2
    # queue order = PE need order: the first V super-wave consumes
    # xT[dt][0:H] + wv[dt] pairs, then the bias tiles, then the xT tails.
    for dt_i in range(DT):
        nc.gpsimd.dma_start(
            out=xT_sb[:, dt_i, 0:H], in_=xT_h[dt_i * P : (dt_i + 1) * P, 0:H]
        )
        nc.gpsimd.dma_start(
            out=wv_t[:, dt_i, :], in_=w_h["v"][dt_i * P : (dt_i + 1) * P, :]
        )
    bvB = const.tile([P, D], F32, tag="bvB", name=f"bvB_{rep}")
    nc.gpsimd.dma_start(out=bvB, in_=_bcast(bv_h[:]))
    bqt_sb = const.tile([P, DT], F32, tag="bqt", name=f"bqt_{rep}")
    nc.gpsimd.dma_start(out=bqt_sb, in_=bqt_h[:])
    bkt_sb = const.tile([P, DT], F32, tag="bkt", name=f"bkt_{rep}")
    nc.gpsimd.dma_start(out=bkt_sb, in_=bkt_h[:])
    for dt_i in range(DT):
        nc.gpsimd.dma_start(
            out=xT_sb[:, dt_i, H:S], in_=xT_h[dt_i * P : (dt_i + 1) * P, H:S]
        )
    for name in ("q", "k", "o"):
        t = const.tile([P, DT, D], BF16, tag=f"w{name}", name=f"w{name}_{rep}")
        for dt_i in range(DT):
            nc.gpsimd.dma_start(
                out=t[:, dt_i, :], in_=w_h[name][dt_i * P : (dt_i + 1) * P, :]
            )
        w_sb[name] = t
    boB = const.tile([P, D], F32, tag="boB", name=f"boB_{rep}")
    nc.gpsimd.dma_start(out=boB, in_=_bcast(bo_h[:]))
    gB = const.tile([P, D], F32, tag="gB", name=f"gB_{rep}")
    nc.gpsimd.dma_start(out=gB, in_=_bcast(g_h[:]))
    bB = const.tile([P, D], F32, tag="bB", name=f"bB_{rep}")
    nc.gpsimd.dma_start(out=bB, in_=_bcast(bb_h[:]))
    onesb = const.tile([P, 1], BF16, tag="ones", name=f"ones_{rep}")
    nc.vector.memset(onesb, 1.0)
    epsc = const.tile([P, 1], F32, tag="eps", name=f"eps_{rep}")
    nc.vector.memset(epsc, EPS)

    QT_sb = qkv.tile([P, DT, QH], BF16, tag="QT", name=f"QT_{rep}")
    KT_sb = qkv.tile([P, DT, S], BF16, tag="KT", name=f"KT_{rep}")
    V_sb = qkv.tile([P, NKT, D], BF16, tag="V", name=f"V_{rep}")

    # ---- projections ----
    # V first, dt-outer over waves of 8 open PSUM banks, so the first
    # matmuls only need wv + xT[d0] (PE starts ~4us after launch instead
    # of waiting for the full xT transfer).
    with tc.tile_pool(name=f"projps{rep}", bufs=8, space="PSUM") as proj_ps:
        for w in range(NKT // 8):
            pss = [
                proj_ps.tile([P, 512], F32, tag="pj", name=f"psv{rep}_{w}_{j}")
                for j in range(8)
            ]
            for dt_i in range(DT):
                for j in range(8):
                    st = w * 8 + j
                    nc.tensor.matmul(
                        pss[j],
                        lhsT=xT_sb[:, dt_i, st * P : (st + 1) * P],
                        rhs=w_sb["v"][:, dt_i, :],
                        start=(dt_i == 0),
                        stop=(dt_i == DT - 1),
                    )
            for j in range(8):
                nc.vector.tensor_add(out=V_sb[:, w * 8 + j, :], in0=pss[j], in1=bvB)
        for et in range(DT):
            for sb_i in range(NQB):
                ps = proj_ps.tile([P, 512], F32, tag="pj", name=f"psq{rep}_{et}_{sb_i}")
                for dt_i in range(DT):
                    nc.tensor.matmul(
                        ps,
                        lhsT=w_sb["q"][:, dt_i, et * P : (et + 1) * P],
                        rhs=xT_sb[:, dt_i, sb_i * 512 : (sb_i + 1) * 512],
                        start=(dt_i == 0),
                        stop=(dt_i == DT - 1),
                    )
                nc.vector.tensor_scalar_add(
                    out=QT_sb[:, et, sb_i * 512 : (sb_i + 1) * 512],
                    in0=ps,
                    scalar1=bqt_sb[:, et : et + 1],
                )
        for et in range(DT):
            for sb_i in range(NSB):
                ps = proj_ps.tile([P, 512], F32, tag="pj", name=f"psk{rep}_{et}_{sb_i}")
                for dt_i in range(DT):
                    nc.tensor.matmul(
                        ps,
                        lhsT=w_sb["k"][:, dt_i, et * P : (et + 1) * P],
                        rhs=xT_sb[:, dt_i, sb_i * 512 : (sb_i + 1) * 512],
                        start=(dt_i == 0),
                        stop=(dt_i == DT - 1),
                    )
                nc.vector.tensor_scalar_add(
                    out=KT_sb[:, et, sb_i * 512 : (sb_i + 1) * 512],
                    in0=ps,
                    scalar1=bkt_sb[:, et : et + 1],
                )

    with (
        tc.tile_pool(name=f"mmps{rep}", bufs=3, space="PSUM") as mm_ps,
        tc.tile_pool(name=f"ups{rep}", bufs=4, space="PSUM") as u_ps,
        tc.tile_pool(name=f"smps{rep}", bufs=1, space="PSUM") as sm_ps,
    ):
        _emit_attention(
            nc, tc, rep,
            (mm_ps, mm_ps, u_ps, sm_ps),
            (expp, op, statp, otp, yp),
            (QT_sb, KT_sb, V_sb, w_sb, onesb, epsc, gB, bB, boB, y_h),
        )


def _emit_attention(nc, tc, rep, psum_pools, sbuf_pools, ctx):
    (mm_ps, yp_ps, u_ps, sm_ps) = psum_pools
    (expp, op, statp, otp, yp) = sbuf_pools
    (QT_sb, KT_sb, V_sb, w_sb, onesb, epsc, gB, bB, boB, y_h) = ctx

    def emit_post_norm(st):
        """Stage 0 of post: reciprocal + normalize all 4 q-slices, freeing
        the U psum banks (and the sums bank) as early as possible."""
        qb, us, sums = st["qb"], st["us"], st["sums"]
        st["OTs"] = [None] * 4
        rT = statp.tile([P, DT], F32, tag="rt", name=f"rt{rep}_{qb}")
        nc.vector.reciprocal(rT, sums)
        st["Os"] = []
        for qs in range(4):
            O = op.tile([P, 512], F32, tag="o", name=f"o{rep}_{qb}_{qs}")
            nc.vector.tensor_scalar_mul(O, in0=us[qs], scalar1=rT[:, qs : qs + 1])
            st["Os"].append(O)

    def emit_post_chain(st, final=False):
        """Batched LN + gelu + transposes for all 4 q-slices: one Sqrt, one
        reciprocal, gelus back-to-back (minimizes ACT function-table
        switches between the k-loop's Exp stream). On the final block the
        ACT engine is idle after the gelus, so the 16 transposes alternate
        between both HWDGE trigger engines to halve the serial pole."""
        qb = st["qb"]
        mv4 = statp.tile([P, 2 * 4], F32, tag="mv", name=f"mv{rep}_{qb}")
        for qs in range(4):
            O = st["Os"][qs]
            st6 = statp.tile([P, 6], F32, tag="bn", name=f"bn{rep}_{qb}_{qs}")
            nc.vector.bn_stats(st6, O)
            nc.vector.bn_aggr(mv4[:, 2 * qs : 2 * qs + 2], st6)
        sd4 = statp.tile([P, 4], F32, tag="sd", name=f"sd{rep}_{qb}")
        nc.scalar.activation(
            out=sd4,
            in_=mv4.rearrange("p (q two) -> p q two", two=2)[:, :, 1],
            func=AF.Sqrt,
            bias=epsc,
        )
        rstd4 = statp.tile([P, 4], F32, tag="rstd", name=f"rstd{rep}_{qb}")
        nc.vector.reciprocal(rstd4, sd4)
        for qs in range(4):
            O = st["Os"][qs]
            nc.vector.tensor_scalar(
                out=O,
                in0=O,
                scalar1=mv4[:, 2 * qs : 2 * qs + 1],
                scalar2=rstd4[:, qs : qs + 1],
                op0=ALU.subtract,
                op1=ALU.mult,
            )
            nc.vector.tensor_mul(O, O, gB)
            nc.vector.tensor_add(O, O, bB)
        Gs = []
        for qs in range(4):
            G = op.tile([P, 512], BF16, tag="g", name=f"g{rep}_{qb}_{qs}")
            nc.scalar.activation(out=G, in_=st["Os"][qs], func=AF.Gelu)
            Gs.append(G)
        for qs in range(4):
            OT = otp.tile([P, DT, P], BF16, tag="ot", name=f"ot{rep}_{qb}_{qs}")
            for i in range(DT):
                eng = nc.scalar if (final and i % 2 == 1) else nc.sync
                eng.dma_start(
                    out=OT[:, i, :],
                    in_=Gs[qs][:, i * P : (i + 1) * P],
                    transpose=True,
                )
            st["OTs"][qs] = OT

    def emit_post_mm(st, qs):
        """out-proj matmuls + bias + writeback for one q-slice."""
        qb = st["qb"]
        OT = st["OTs"][qs]
        if True:
            yps = yp_ps.tile([P, 512], F32, tag="mm", name=f"yps{rep}_{qb}_{qs}")
            for i in range(DT):
                nc.tensor.matmul(
                    yps,
                    lhsT=OT[:, i, :],
                    rhs=w_sb["o"][:, i, :],
                    start=(i == 0),
                    stop=(i == DT - 1),
                )
            Y = yp.tile([P, 512], F32, tag="yo", name=f"y{rep}_{qb}_{qs}")
            nc.vector.tensor_add(Y, yps, boB)
            row = (qb * 4 + qs) * P
            nc.gpsimd.dma_start(out=y_h[row : row + P, :], in_=Y)

    pending = None
    for qb in range(NQB):
        us = [
            u_ps.tile([P, 512], F32, tag="u", name=f"u{rep}_{qb}_{i}")
            for i in range(4)
        ]
        sums = sm_ps.tile([P, DT], F32, tag="sums", name=f"sums{rep}_{qb}")

        exhist = {}
        for kt in range(NKT + 2):
            if kt < NKT:
                sps = mm_ps.tile([P, 512], F32, tag="mm", name=f"s{rep}_{qb}_{kt}")
                for et in range(DT):
                    nc.tensor.matmul(
                        sps,
                        lhsT=KT_sb[:, et, kt * P : (kt + 1) * P],
                        rhs=QT_sb[:, et, qb * 512 : (qb + 1) * 512],
                        start=(et == 0),
                        stop=(et == DT - 1),
                    )
                ex = expp.tile([P, 512], BF16, tag="ex", name=f"ex{rep}_{qb}_{kt}")
                nc.scalar.activation(out=ex, in_=sps, func=AF.Exp, scale=SCALE)
                exhist[kt] = ex
            if kt >= 2:
                kp = kt - 2
                ex_use = exhist.pop(kp)
                for qs in range(4):
                    lhs = ex_use[:, qs * P : (qs + 1) * P]
                    nc.tensor.matmul(
                        us[qs],
                        lhsT=lhs,
                        rhs=V_sb[:, kp, :],
                        start=(kp == 0),
                        stop=(kp == NKT - 1),
                    )
                    # same stationary as the PV matmul just above; all 4
                    # sums columns share one psum zero region.
                    nc.tensor.matmul(
                        sums[:, qs : qs + 1],
                        lhsT=lhs,
                        rhs=onesb,
                        start=(kp == 0 and qs == 0),
                        stop=(kp == NKT - 1 and qs == 3),
                        skip_group_check=True,
                    )
            if pending is not None:
                if kt == POST_KT:
                    emit_post_norm(pending)
                if kt == POST_KT + 2:
                    emit_post_chain(pending)
                for j in range(4):
                    if kt == POST_KT + 7 + 3 * j:
                        emit_post_mm(pending, j)
                        if j == 3:
                            pending = None
        pending = {"qb": qb, "us": us, "sums": sums, "OTs": [None] * 4}
    emit_post_norm(pending)
    emit_post_chain(pending, final=True)
    for j in range(4):
        emit_post_mm(pending, j)


def _build(repeat=1):
    nc = bacc.Bacc(None, target_bir_lowering=False, num_swdge_queues=4)

    xT_h = nc.dram_tensor("xT", [D, S], BF16, kind="ExternalInput")
    w_h = {
        "q": nc.dram_tensor("wqT", [D, D], BF16, kind="ExternalInput"),
        "k": nc.dram_tensor("wkT", [D, D], BF16, kind="ExternalInput"),
        "v": nc.dram_tensor("wvT", [D, D], BF16, kind="ExternalInput"),
        "o": nc.dram_tensor("woT", [D, D], BF16, kind="ExternalInput"),
    }
    bqt_h = nc.dram_tensor("bqt", [P, DT], F32, kind="ExternalInput")
    bkt_h = nc.dram_tensor("bkt", [P, DT], F32, kind="ExternalInput")
    bv_h = nc.dram_tensor("bv_v", [D], F32, kind="ExternalInput")
    bo_h = nc.dram_tensor("bo_v", [D], F32, kind="ExternalInput")
    g_h = nc.dram_tensor("g_v", [D], F32, kind="ExternalInput")
    bb_h = nc.dram_tensor("b_v", [D], F32, kind="ExternalInput")
    y_h = nc.dram_tensor("y", [QH, D], F32, kind="ExternalOutput")
    handles = (xT_h, w_h, bqt_h, bkt_h, bv_h, bo_h, g_h, bb_h, y_h)

    with tile.TileContext(nc) as tc:
        for rep in range(repeat):
            with (
                tc.tile_pool(name=f"const{rep}", bufs=1) as const,
                tc.tile_pool(name=f"qkv{rep}", bufs=1) as qkv,
                tc.tile_pool(name=f"expp{rep}", bufs=10) as expp,
                tc.tile_pool(name=f"op{rep}", bufs=8) as op,
                tc.tile_pool(name=f"stat{rep}", bufs=6) as statp,
                tc.tile_pool(name=f"otp{rep}", bufs=8) as otp,
                tc.tile_pool(name=f"yp{rep}", bufs=4) as yp,
            ):
                pools = (const, qkv, expp, op, statp, otp, yp)
                _emit_body(nc, tc, pools, handles, rep)

    nc.finalize()
    return nc


def kernel(**inputs):
    global _cached_nc, last_results
    x = np.asarray(inputs["x"], dtype=np.float32)
    f32 = lambda k: np.ascontiguousarray(np.asarray(inputs[k], dtype=np.float32))
    wT = {
        k: np.ascontiguousarray(np.asarray(inputs[k], dtype=np.float32).T).astype(
            ml_dtypes.bfloat16
        )
        for k in ("Wq", "Wk", "Wv", "Wo")
    }
    bqt = np.ascontiguousarray(f32("bq").reshape(DT, P).T)
    bkt = np.ascontiguousarray(f32("bk").reshape(DT, P).T)
    bv, bo, g, bb = f32("bv"), f32("bo"), f32("ln_g"), f32("ln_b")

    if _cached_nc is None:
        _cached_nc = _build()
    nc = _cached_nc

    in_maps = []
    for c in range(N_CORES):
        bi, h = divmod(c, 2)
        xr = x[bi] if h == 0 else np.roll(x[bi], -QH, axis=0)
        xT = np.ascontiguousarray(xr.T).astype(ml_dtypes.bfloat16)
        in_maps.append(
            dict(
                xT=xT,
                wqT=wT["Wq"],
                wkT=wT["Wk"],
                wvT=wT["Wv"],
                woT=wT["Wo"],
                bqt=bqt,
                bkt=bkt,
                bv_v=bv,
                bo_v=bo,
                g_v=g,
                b_v=bb,
            )
        )

    res = run_bass_kernel_spmd(
        nc, in_maps, core_ids=list(range(N_CORES)), trace=TRACE, **TRACE_KW
    )
    last_results = res

    out = np.empty((B, S, D), dtype=np.float32)
    for c in range(N_CORES):
        bi, h = divmod(c, 2)
        out[bi, h * QH : (h + 1) * QH] = res.results[c]["y"]
    return out



# revision 2
# speedup vs baseline: 12086.5840x; 12086.5840x over previous
"""Trainium2 Bass kernel for nn_ExpertHead: full attention head.

Reference computation (per batch b):
    Q = x Wq^T + bq; K = x Wk^T + bk; V = x Wv^T + bv        [S, D]
    P = softmax(Q K^T / sqrt(D))                              [S, S]
    O = layernorm(P V) -> gelu(exact) -> O Wo^T + bo          [S, D]

Sharding: 8 cores, B=4 batches -> each core handles one half (2048 rows)
of one batch's queries, with full K/V for that batch computed locally
(weights replicated). No collectives. The host rolls x so each core's
queries are always the first QH rows (softmax/PV are permutation
invariant over keys).

Layout strategy (per core):
  - host passes x^T (d-major) in bf16, plus transposed bf16 weights, so
    every matmul contraction dim is already on partitions; no on-chip
    transposes of activations are needed except the post-gelu tiles,
    which use the DMA xbar transpose (bf16).
  - scores are computed TRANSPOSED: S^T[k, q], so exp(S^T) feeds the PV
    matmul directly as the stationary operand.
  - softmax row sums land directly in per-partition-scalar layout via
    matmul(lhsT=exp_block, rhs=ones[128,1]) -> [q, 1]; all 4 q-slice
    sums accumulate in one PSUM bank (one zero region: start only on
    the very first MM, stop on the very last).
  - post-processing (normalize, LN, gelu, transpose, out-proj) of query
    block qb is emitted inside the k-loop of block qb+1 so the PE never
    waits on the DVE/ACT chain.
"""

import numpy as np
import ml_dtypes

import concourse.bass as bass
import concourse.mybir as mybir
import concourse.tile as tile
from concourse import bacc
from concourse.bass_utils import run_bass_kernel_spmd

BF16 = mybir.dt.bfloat16
F32 = mybir.dt.float32
AF = mybir.ActivationFunctionType
ALU = mybir.AluOpType

B, S, D = 4, 4096, 512
P = 128
QH = S // 2          # queries per core
DT = D // P          # 4 contraction tiles of 128
NKT = S // P         # 32 key tiles
NQB = QH // 512      # 4 query blocks of 512
NSB = S // 512       # 8 s blocks of 512
SCALE = float(1.0 / np.sqrt(np.float32(D)))
EPS = 1e-5
N_CORES = 8
POST_KT = 6          # k-iteration of next block where prev post is emitted

TRACE = False
TRACE_KW = {}
last_results = None

_cached_nc = None


def _bcast(ap1d, parts=P):
    """[N] dram AP -> [parts, N] partition-broadcast AP (step 0)."""
    return bass.AP(
        tensor=ap1d.tensor,
        offset=ap1d.offset,
        ap=[[0, parts], list(ap1d.ap[0])],
    )


def _emit_body(nc, tc, ctxpools, handles, rep):
    (xT_h, w_h, bqt_h, bkt_h, bv_h, bo_h, g_h, bb_h, y_h) = handles
    (const, qkv, expp, op, statp, otp, yp) = ctxpools

    # ---- constants / weights into SBUF.
    # DMA queue order matters for PE start latency (SWDGE trigger latency
    # ~1us per transfer): first the xT-half/wv pairs the V projection
    # consumes, then projection-phase biases, then everything else.
    w_sb = {}
    wv_t = const.tile([P, DT, D], BF16, tag="wv", name=f"wv_{rep}")
    w_sb["v"] = wv_t
    xT_sb = const.tile([P, DT, S], BF16, tag="xT", name=f"xT_{rep}")
    H = S // 2
    # queue order = PE need order: the first V super-wave consumes
    # xT[dt][0:H] + wv[dt] pairs, then the bias tiles, then the xT tails.
    for dt_i in range(DT):
        nc.gpsimd.dma_start(
            out=xT_sb[:, dt_i, 0:H], in_=xT_h[dt_i * P : (dt_i + 1) * P, 0:H]
        )
        nc.gpsimd.dma_start(
            out=wv_t[:, dt_i, :], in_=w_h["v"][dt_i * P : (dt_i + 1) * P, :]
        )
    bvB = const.tile([P, D], F32, tag="bvB", name=f"bvB_{rep}")
    nc.gpsimd.dma_start(out=bvB, in_=_bcast(bv_h[:]))
    bqt_sb = const.tile([P, DT], F32, tag="bqt", name=f"bqt_{rep}")
    nc.gpsimd.dma_start(out=bqt_sb, in_=bqt_h[:])
    bkt_sb = const.tile([P, DT], F32, tag="bkt", name=f"bkt_{rep}")
    nc.gpsimd.dma_start(out=bkt_sb, in_=bkt_h[:])
    for dt_i in range(DT):
        nc.gpsimd.dma_start(
            out=xT_sb[:, dt_i, H:S], in_=xT_h[dt_i * P : (dt_i + 1) * P, H:S]
        )
    for name in ("q", "k", "o"):
        t = const.tile([P, DT, D], BF16, tag=f"w{name}", name=f"w{name}_{rep}")
        for dt_i in range(DT):
            nc.gpsimd.dma_start(
                out=t[:, dt_i, :], in_=w_h[name][dt_i * P : (dt_i + 1) * P, :]
            )
        w_sb[name] = t
    boB = const.tile([P, D], F32, tag="boB", name=f"boB_{rep}")
    nc.gpsimd.dma_start(out=boB, in_=_bcast(bo_h[:]))
    gB = const.tile([P, D], F32, tag="gB", name=f"gB_{rep}")
    nc.gpsimd.dma_start(out=gB, in_=_bcast(g_h[:]))
    bB = const.tile([P, D], F32, tag="bB", name=f"bB_{rep}")
    nc.gpsimd.dma_start(out=bB, in_=_bcast(bb_h[:]))
    onesb = const.tile([P, 1], BF16, tag="ones", name=f"ones_{rep}")
    nc.vector.memset(onesb, 1.0)
    epsc = const.tile([P, 1], F32, tag="eps", name=f"eps_{rep}")
    nc.vector.memset(epsc, EPS)

    QT_sb = qkv.tile([P, DT, QH], BF16, tag="QT", name=f"QT_{rep}")
    KT_sb = qkv.tile([P, DT, S], BF16, tag="KT", name=f"KT_{rep}")
    V_sb = qkv.tile([P, NKT, D], BF16, tag="V", name=f"V_{rep}")

    # ---- projections ----
    # V first, dt-outer over waves of 8 open PSUM banks, so the first
    # matmuls only need wv + xT[d0] (PE starts ~4us after launch instead
    # of waiting for the full xT transfer).
    with tc.tile_pool(name=f"projps{rep}", bufs=8, space="PSUM") as proj_ps:
        for w in range(NKT // 8):
            pss = [
                proj_ps.tile([P, 512], F32, tag="pj", name=f"psv{rep}_{w}_{j}")
                for j in range(8)
            ]
            for dt_i in range(DT):
                for j in range(8):
                    st = w * 8 + j
                    nc.tensor.matmul(
                        pss[j],
                        lhsT=xT_sb[:, dt_i, st * P : (st + 1) * P],
                        rhs=w_sb["v"][:, dt_i, :],
                        start=(dt_i == 0),
                        stop=(dt_i == DT - 1),
                    )
            for j in range(8):
                nc.vector.tensor_add(out=V_sb[:, w * 8 + j, :], in0=pss[j], in1=bvB)
        for et in range(DT):
            for sb_i in range(NQB):
                ps = proj_ps.tile([P, 512], F32, tag="pj", name=f"psq{rep}_{et}_{sb_i}")
                for dt_i in range(DT):
                    nc.tensor.matmul(
                        ps,
                        lhsT=w_sb["q"][:, dt_i, et * P : (et + 1) * P],
                        rhs=xT_sb[:, dt_i, sb_i * 512 : (sb_i + 1) * 512],
                        start=(dt_i == 0),
                        stop=(dt_i == DT - 1),
                    )
                nc.vector.tensor_scalar_add(
                    out=QT_sb[:, et, sb_i * 512 : (sb_i + 1) * 512],
                    in0=ps,
                    scalar1=bqt_sb[:, et : et + 1],
                )
        for et in range(DT):
            for sb_i in range(NSB):
                ps = proj_ps.tile([P, 512], F32, tag="pj", name=f"psk{rep}_{et}_{sb_i}")
                for dt_i in range(DT):
                    nc.tensor.matmul(
                        ps,
                        lhsT=w_sb["k"][:, dt_i, et * P : (et + 1) * P],
                        rhs=xT_sb[:, dt_i, sb_i * 512 : (sb_i + 1) * 512],
                        start=(dt_i == 0),
                        stop=(dt_i == DT - 1),
                    )
                nc.vector.tensor_scalar_add(
                    out=KT_sb[:, et, sb_i * 512 : (sb_i + 1) * 512],
                    in0=ps,
                    scalar1=bkt_sb[:, et : et + 1],
                )

    with (
        tc.tile_pool(name=f"mmps{rep}", bufs=3, space="PSUM") as mm_ps,
        tc.tile_pool(name=f"ups{rep}", bufs=4, space="PSUM") as u_ps,
        tc.tile_pool(name=f"smps{rep}", bufs=1, space="PSUM") as sm_ps,
    ):
        _emit_attention(
            nc, tc, rep,
            (mm_ps, mm_ps, u_ps, sm_ps),
            (expp, op, statp, otp, yp),
            (QT_sb, KT_sb, V_sb, w_sb, onesb, epsc, gB, bB, boB, y_h),
        )


def _emit_attention(nc, tc, rep, psum_pools, sbuf_pools, ctx):
    (mm_ps, yp_ps, u_ps, sm_ps) = psum_pools
    (expp, op, statp, otp, yp) = sbuf_pools
    (QT_sb, KT_sb, V_sb, w_sb, onesb, epsc, gB, bB, boB, y_h) = ctx

    def emit_post_norm(st):
        """Stage 0 of post: reciprocal + normalize all 4 q-slices, freeing
        the U psum banks (and the sums bank) as early as possible."""
        qb, us, sums = st["qb"], st["us"], st["sums"]
        st["OTs"] = [None] * 4
        rT = statp.tile([P, DT], F32, tag="rt", name=f"rt{rep}_{qb}")
        nc.vector.reciprocal(rT, sums)
        st["Os"] = []
        for qs in range(4):
            O = op.tile([P, 512], F32, tag="o", name=f"o{rep}_{qb}_{qs}")
            nc.vector.tensor_scalar_mul(O, in0=us[qs], scalar1=rT[:, qs : qs + 1])
            st["Os"].append(O)

    def emit_post_chain(st, final=False):
        """Batched LN + gelu + transposes for all 4 q-slices: one Sqrt, one
        reciprocal, gelus back-to-back (minimizes ACT function-table
        switches between the k-loop's Exp stream). On the final block the
        ACT engine is idle after the gelus, so the 16 transposes alternate
        between both HWDGE trigger engines to halve the serial pole."""
        qb = st["qb"]
        mv4 = statp.tile([P, 2 * 4], F32, tag="mv", name=f"mv{rep}_{qb}")
        for qs in range(4):
            O = st["Os"][qs]
            st6 = statp.tile([P, 6], F32, tag="bn", name=f"bn{rep}_{qb}_{qs}")
            nc.vector.bn_stats(st6, O)
            nc.vector.bn_aggr(mv4[:, 2 * qs : 2 * qs + 2], st6)
        sd4 = statp.tile([P, 4], F32, tag="sd", name=f"sd{rep}_{qb}")
        nc.scalar.activation(
            out=sd4,
            in_=mv4.rearrange("p (q two) -> p q two", two=2)[:, :, 1],
            func=AF.Sqrt,
            bias=epsc,
        )
        rstd4 = statp.tile([P, 4], F32, tag="rstd", name=f"rstd{rep}_{qb}")
        nc.vector.reciprocal(rstd4, sd4)
        for qs in range(4):
            O = st["Os"][qs]
            nc.vector.tensor_scalar(
                out=O,
                in0=O,
                scalar1=mv4[:, 2 * qs : 2 * qs + 1],
                scalar2=rstd4[:, qs : qs + 1],
                op0=ALU.subtract,
                op1=ALU.mult,
            )
            nc.vector.tensor_mul(O, O, gB)
            nc.vector.tensor_add(O, O, bB)
        Gs = []
        for qs in range(4):
            G = op.tile([P, 512], BF16, tag="g", name=f"g{rep}_{qb}_{qs}")
            nc.scalar.activation(out=G, in_=st["Os"][qs], func=AF.Gelu)
            Gs.append(G)
        for qs in range(4):
            OT = otp.tile([P, DT, P], BF16, tag="ot", name=f"ot{rep}_{qb}_{qs}")
            for i in range(DT):
                eng = nc.scalar if (final and i % 2 == 1) else nc.sync
                eng.dma_start(
                    out=OT[:, i, :],
                    in_=Gs[qs][:, i * P : (i + 1) * P],
                    transpose=True,
                )
            st["OTs"][qs] = OT

    def emit_post_mm(st, qs):
        """out-proj matmuls + bias + writeback for one q-slice."""
        qb = st["qb"]
        OT = st["OTs"][qs]
        if True:
            yps = yp_ps.tile([P, 512], F32, tag="mm", name=f"yps{rep}_{qb}_{qs}")
            for i in range(DT):
                nc.tensor.matmul(
                    yps,
                    lhsT=OT[:, i, :],
                    rhs=w_sb["o"][:, i, :],
                    start=(i == 0),
                    stop=(i == DT - 1),
                )
            Y = yp.tile([P, 512], F32, tag="yo", name=f"y{rep}_{qb}_{qs}")
            nc.vector.tensor_add(Y, yps, boB)
            row = (qb * 4 + qs) * P
            nc.gpsimd.dma_start(out=y_h[row : row + P, :], in_=Y)

    pending = None
    for qb in range(NQB):
        us = [
            u_ps.tile([P, 512], F32, tag="u", name=f"u{rep}_{qb}_{i}")
            for i in range(4)
        ]
        sums = sm_ps.tile([P, DT], F32, tag="sums", name=f"sums{rep}_{qb}")

        exhist = {}
        for kt in range(NKT + 2):
            if kt < NKT:
                sps = mm_ps.tile([P, 512], F32, tag="mm", name=f"s{rep}_{qb}_{kt}")
                for et in range(DT):
                    nc.tensor.matmul(
                        sps,
                        lhsT=KT_sb[:, et, kt * P : (kt + 1) * P],
                        rhs=QT_sb[:, et, qb * 512 : (qb + 1) * 512],
                        start=(et == 0),
                        stop=(et == DT - 1),
                    )
                ex = expp.tile([P, 512], BF16, tag="ex", name=f"ex{rep}_{qb}_{kt}")
                nc.scalar.activation(out=ex, in_=sps, func=AF.Exp, scale=SCALE)
                exhist[kt] = ex
            if kt >= 2:
                kp = kt - 2
                ex_use = exhist.pop(kp)
                for qs in range(4):
                    lhs = ex_use[:, qs * P : (qs + 1) * P]
                    nc.tensor.matmul(
                        us[qs],
                        lhsT=lhs,
                        rhs=V_sb[:, kp, :],
                        start=(kp == 0),
                        stop=(kp == NKT - 1),
                    )
                    # same stationary as the PV matmul just above; all 4
                    # sums columns share one psum zero region.
                    nc.tensor.matmul(
                        sums[:, qs : qs + 1],
                        lhsT=lhs,
                        rhs=onesb,
                        start=(kp == 0 and qs == 0),
                        stop=(kp == NKT - 1 and qs == 3),
                        skip_group_check=True,
                    )
            if pending is not None:
                if kt == POST_KT:
                    emit_post_norm(pending)
                if kt == POST_KT + 2:
                    emit_post_chain(pending)
                for j in range(4):
                    if kt == POST_KT + 7 + 3 * j:
                        emit_post_mm(pending, j)
                        if j == 3:
                            pending = None
        pending = {"qb": qb, "us": us, "sums": sums, "OTs": [None] * 4}
    emit_post_norm(pending)
    emit_post_chain(pending, final=True)
    for j in range(4):
        emit_post_mm(pending, j)


def _build(repeat=1):
    nc = bacc.Bacc(None, target_bir_lowering=False, num_swdge_queues=4)

    xT_h = nc.dram_tensor("xT", [D, S], BF16, kind="ExternalInput")
    w_h = {
        "q": nc.dram_tensor("wqT", [D, D], BF16, kind="ExternalInput"),
        "k": nc.dram_tensor("wkT", [D, D], BF16, kind="ExternalInput"),
        "v": nc.dram_tensor("wvT", [D, D], BF16, kind="ExternalInput"),
        "o": nc.dram_tensor("woT", [D, D], BF16, kind="ExternalInput"),
    }
    bqt_h = nc.dram_tensor("bqt", [P, DT], F32, kind="ExternalInput")
    bkt_h = nc.dram_tensor("bkt", [P, DT], F32, kind="ExternalInput")
    bv_h = nc.dram_tensor("bv_v", [D], F32, kind="ExternalInput")
    bo_h = nc.dram_tensor("bo_v", [D], F32, kind="ExternalInput")
    g_h = nc.dram_tensor("g_v", [D], F32, kind="ExternalInput")
    bb_h = nc.dram_tensor("b_v", [D], F32, kind="ExternalInput")
    y_h = nc.dram_tensor("y", [QH, D], F32, kind="ExternalOutput")
    handles = (xT_h, w_h, bqt_h, bkt_h, bv_h, bo_h, g_h, bb_h, y_h)

    with tile.TileContext(nc) as tc:
        for rep in range(repeat):
            with (
                tc.tile_pool(name=f"const{rep}", bufs=1) as const,
                tc.tile_pool(name=f"qkv{rep}", bufs=1) as qkv,
                tc.tile_pool(name=f"expp{rep}", bufs=10) as expp,
                tc.tile_pool(name=f"op{rep}", bufs=8) as op,
                tc.tile_pool(name=f"stat{rep}", bufs=6) as statp,
                tc.tile_pool(name=f"otp{rep}", bufs=8) as otp,
                tc.tile_pool(name=f"yp{rep}", bufs=4) as yp,
            ):
                pools = (const, qkv, expp, op, statp, otp, yp)
                _emit_body(nc, tc, pools, handles, rep)

    nc.finalize()
    return nc


def prepare_in_maps(inputs):
    x = np.asarray(inputs["x"], dtype=np.float32)
    f32 = lambda k: np.ascontiguousarray(np.asarray(inputs[k], dtype=np.float32))
    wT = {
        k: np.ascontiguousarray(np.asarray(inputs[k], dtype=np.float32).T).astype(
            ml_dtypes.bfloat16
        )
        for k in ("Wq", "Wk", "Wv", "Wo")
    }
    bqt = np.ascontiguousarray(f32("bq").reshape(DT, P).T)
    bkt = np.ascontiguousarray(f32("bk").reshape(DT, P).T)
    bv, bo, g, bb = f32("bv"), f32("bo"), f32("ln_g"), f32("ln_b")

    in_maps = []
    for c in range(N_CORES):
        bi, h = divmod(c, 2)
        xr = x[bi] if h == 0 else np.roll(x[bi], -QH, axis=0)
        xT = np.ascontiguousarray(xr.T).astype(ml_dtypes.bfloat16)
        in_maps.append(
            dict(
                xT=xT,
                wqT=wT["Wq"],
                wkT=wT["Wk"],
                wvT=wT["Wv"],
                woT=wT["Wo"],
                bqt=bqt,
                bkt=bkt,
                bv_v=bv,
                bo_v=bo,
                g_v=g,
                b_v=bb,
            )
        )
    return in_maps


def kernel(**inputs):
    global _cached_nc, last_results
    if _cached_nc is None:
        _cached_nc = _build()
    nc = _cached_nc
    in_maps = prepare_in_maps(inputs)

    res = run_bass_kernel_spmd(
        nc, in_maps, core_ids=list(range(N_CORES)), trace=TRACE, **TRACE_KW
    )
    last_results = res

    out = np.empty((B, S, D), dtype=np.float32)
    for c in range(N_CORES):
        bi, h = divmod(c, 2)
        out[bi, h * QH : (h + 1) * QH] = res.results[c]["y"]
    return out

